# revision 1
# baseline (speedup 1.0000x reference)
"""E3Conv Trainium2 kernel: 8-core SPMD, dst-partitioned edges.

Strategy: sort edges by dst; core i owns nodes [1250i,1250(i+1)) and all edges
into them (no all-reduce needed). Per core: node-MLP replicated, SBUF-resident
bf16 gather table for Ai (recip folded into one-hot scatter weights), radial
MLP + tensor-product as K=512 matmuls per edge tile, PSUM-accumulated
one-hot matmul scatter-mean. Engine balance: PE matmuls, DVE fused
PSUM-multiply TTs, Act silu/copies, Pool gathers.
"""
import sys, os
sys.path.insert(0, "/opt/trn_rl_repo")
import numpy as np

import concourse.bass as bass
import concourse.tile as tile
from concourse import bacc, mybir
from concourse import bass_utils
from concourse.masks import make_identity

P = 128
N_NODES, N_EDGES, N_GRAPHS = 10000, 131072, 64
N_CORES, NPC, N_WIN = 8, 1250, 10
MAX_RADIUS, N_BASIS = 4.0, 10
STEP = MAX_RADIUS / (N_BASIS + 1)
VCENTERS = np.linspace(0.0, MAX_RADIUS, N_BASIS + 2)[1:-1].astype(np.float32)
F32, BF16, I16 = mybir.dt.float32, mybir.dt.bfloat16, mybir.dt.int16
AF = mybir.ActivationFunctionType
ALU = mybir.AluOpType
NCH = 79  # node chunks of 128 (79*128 = 10112 >= 10000)
NF = 19   # geometry features: 10 basis + 3 sh1(u) + 5 sh2 + ones


def _build_consts(fW4):
    s3 = 3.0 ** 0.5
    W4p = np.zeros((512, 224), np.float32)
    offs = {0: 0, 1: 1024, 2: 1536}
    wbase = {0: 0, 1: 16, 2: 24}
    scale_l = {0: 1.0 / 64, 1: s3 / 64, 2: 1.0 / 64}
    for l, mo in enumerate((16, 8, 4)):
        for u in range(8):
            for v in range(8):
                for wl in range(mo):
                    col = offs[l] + (u * 8 + v) * mo + wl
                    w = wbase[l] + wl
                    W4p[np.arange(64) * 8 + v, w * 8 + u] = fW4[:, col] * scale_l[l]
    Sel = np.zeros((4, 64, 128), np.float32)
    for q in range(4):
        for r in range(128):
            Sel[q, 16 * q + r // 8, r] = 1.0
    # L2A: tm0 (w=0..13) -> l0 slots 0..13 ; L2B: tm1 (w=14..27) ->
    # l0 slots 14,15 ; l1 slots 16+(w-16)*3+m ; l2 slots 40+(w-24)*5+k
    L2A = np.zeros((112, 60), np.float32)
    L2B = np.zeros((112, 60), np.float32)
    for r in range(112):
        L2A[r, r // 8] = 1.0
        w = 14 + r // 8
        if w < 16:
            L2B[r, w] = 1.0
        elif w < 24:
            for m in range(3):
                L2B[r, 16 + (w - 16) * 3 + m] = 1.0
        else:
            for k in range(5):
                L2B[r, 40 + (w - 24) * 5 + k] = 1.0
    # HSb: bt19 rows (10 basis, u xyz, 5 sh2, ones) -> shs rows
    # [0:16 ones | 16:40 l1 = u comps | 40:60 l2 comps]
    HSb = np.zeros((19, 60), np.float32)
    HSb[18, 0:16] = 1.0
    for w in range(8):
        for m in range(3):
            HSb[10 + m, 16 + w * 3 + m] = 1.0
    for w in range(4):
        for k in range(5):
            HSb[13 + k, 40 + w * 5 + k] = 1.0
    return W4p, Sel, L2A, L2B, HSb


def _merge_hs(HSb, fW1p):
    HS = np.zeros((19, 124), np.float32)
    HS[0:10, 0:64] = fW1p
    HS[:, 64:124] = HSb
    return HS


def _host_prep(inputs):
    pos = np.asarray(inputs["pos"], np.float32)
    A = np.asarray(inputs["A"]).astype(np.int64)
    batch = np.asarray(inputs["batch"]).astype(np.int64)
    esrc = np.asarray(inputs["edge_src"]).astype(np.int64)
    edst = np.asarray(inputs["edge_dst"]).astype(np.int64)
    shifts = np.asarray(inputs["edge_shifts"], np.float32)
    cell = np.asarray(inputs["cell"], np.float32)
    counts = np.bincount(edst, minlength=N_NODES).astype(np.float32)
    recipc = 1.0 / np.maximum(counts, 1.0)
    cpn = cell[batch].reshape(N_NODES, 9)
    order = np.argsort(edst, kind="stable")
    wins_all, W_CH = [], 0
    for ci in range(N_CORES):
        lo = ci * NPC
        m = order[(edst[order] >= lo) & (edst[order] < lo + NPC)]
        wins = []
        for w in range(N_WIN):
            wlo = lo + w * P
            whi = min(lo + (w + 1) * P, lo + NPC)
            wm = m[(edst[m] >= wlo) & (edst[m] < whi)]
            wins.append(wm)
            W_CH = max(W_CH, (len(wm) + P - 1) // P)
        wins_all.append(wins)
    if W_CH % 2:
        W_CH += 1
    C_TOT = N_WIN * W_CH
    E = C_TOT * P
    onehotA = np.zeros((10, NCH * P), np.float32)
    onehotA[A, np.arange(N_NODES)] = 1.0
    recip_pad = np.concatenate([recipc, np.ones(N_WIN * P * N_CORES, np.float32)])
    per_core = []
    for ci in range(N_CORES):
        idx = np.zeros(E, np.int64)
        valid = np.zeros(E, bool)
        dstloc = np.full(E, -1.0, np.float32)
        for w in range(N_WIN):
            wm = wins_all[ci][w]
            s = w * W_CH * P
            idx[s:s + len(wm)] = wm
            valid[s:s + len(wm)] = True
            dstloc[s:s + len(wm)] = (edst[wm] - ci * NPC - w * P).astype(np.float32)
        src = np.where(valid, esrc[idx], 0)
        dst = np.where(valid, edst[idx], 0)
        sh = np.where(valid[:, None], shifts[idx], np.float32(1.0))
        geom = np.concatenate([pos[src], pos[dst], sh, cpn[src]], 1)  # [E,18]
        geom_pl = np.ascontiguousarray(
            np.transpose(geom.reshape(C_TOT, P, 18), (1, 2, 0)).reshape(P, 18 * C_TOT))

        def wrap(ix):
            wr = ix.astype(np.int16).reshape(-1, 16).T  # [16, E/16]
            return np.ascontiguousarray(np.tile(wr, (8, 1)))
        # one-hot scatter weights carry the scatter-mean reciprocal
        ohm = (dstloc.reshape(C_TOT, P, 1) ==
               np.arange(P, dtype=np.float32)[None, None, :]).astype(np.float32)
        for w in range(N_WIN):
            rw = recip_pad[ci * NPC + w * P: ci * NPC + (w + 1) * P]
            ohm[w * W_CH:(w + 1) * W_CH] *= rw[None, None, :]
        oh_pl = np.ascontiguousarray(
            np.transpose(ohm, (1, 0, 2)).reshape(P, C_TOT * P))
        per_core.append(dict(geom_pl=geom_pl, oh_pl=oh_pl,
                             idx_src=wrap(src), idx_dst=wrap(dst)))
    return per_core, onehotA, W_CH, C_TOT, E


def _build_bass(W_CH, C_TOT, E, consts):
    TILE_CH = W_CH // 2
    NT = C_TOT // TILE_CH
    ET = TILE_CH * P
    NIW = E // 16
    nc = bacc.Bacc("TRN2", target_bir_lowering=False, debug=False,
                   num_devices=N_CORES)

    def din(name, shape, dt=F32):
        return nc.dram_tensor(name, shape, dt, kind="ExternalInput").ap()

    geom_d = din("geom_pl", [P, 18 * C_TOT])
    ohm_d = din("oh_pl", [P, C_TOT * P], BF16)
    isrc_d = din("idx_src", [P, NIW], I16)
    idst_d = din("idx_dst", [P, NIW], I16)
    ohA_d = din("onehotA", [10, NCH * P], BF16)
    TA_d = din("TA", [64, 10], BF16)
    W2_d = din("fit_W2", [64, 32], BF16)
    W3_d = din("fit_W3", [32, 8], BF16)
    HS_d = din("HS", [19, 124], BF16)
    fW2_d = din("fc_W2p", [64, 64], BF16)
    fW3_d = din("fc_W3p", [64, 4 * 128], BF16)
    W4p_d = din("W4p", [128, 4 * 224], BF16)
    L2A_d = din("L2A", [112, 60], BF16)
    L2B_d = din("L2B", [112, 60], BF16)
    cv_d = din("cvec", [P, 16])
    out_d = nc.dram_tensor("out", [N_WIN * P, 60], F32, kind="ExternalOutput").ap()

    C = C_TOT
    with tile.TileContext(nc) as tc:
        with tc.tile_pool(name="const", bufs=1) as cp, \
             tc.tile_pool(name="sb", bufs=2) as sp, \
             tc.tile_pool(name="big", bufs=1) as bp, \
             tc.tile_pool(name="ps", bufs=2, space="PSUM") as ps, \
             tc.tile_pool(name="pc", bufs=1, space="PSUM") as pc, \
             tc.tile_pool(name="pf", bufs=1, space="PSUM") as pf, \
             tc.tile_pool(name="pw", bufs=1, space="PSUM") as pw:
            ident = cp.tile([P, P], F32)
            make_identity(nc, ident[:])
            identb = cp.tile([P, P], BF16)
            nc.vector.tensor_copy(identb[:], ident[:])

            def load_const(dram, shape, dt=F32):
                t = cp.tile(shape, dt, tag=dram.tensor.name)
                nc.sync.dma_start(t[:], dram[:])
                return t
            TA = load_const(TA_d, [64, 10], BF16)
            W2 = load_const(W2_d, [64, 32], BF16)
            W3 = load_const(W3_d, [32, 8], BF16)
            HSt = load_const(HS_d, [19, 124], BF16)
            fW2 = load_const(fW2_d, [64, 64], BF16)
            fW3 = load_const(fW3_d, [64, 4 * 128], BF16)
            W4pt = load_const(W4p_d, [128, 4 * 224], BF16)
            L2At = load_const(L2A_d, [112, 60], BF16)
            L2Bt = load_const(L2B_d, [112, 60], BF16)
            cv = load_const(cv_d, [P, 16])
            ohA = bp.tile([10, NCH * P], BF16)
            nc.sync.dma_start(ohA[:], ohA_d[:])
            isrc = bp.tile([P, NIW], I16)
            nc.sync.dma_start(isrc[:], isrc_d[:])
            idst = bp.tile([P, NIW], I16)
            nc.sync.dma_start(idst[:], idst_d[:])

            # ---- node MLP degenerates to a 10-row type table (input depends
            # only on atom type); expand per 128-node chunk into the SBUF
            # gather table (node n -> partition n%128, rank n//128, x16) ----
            s1 = sp.tile([64, 10], BF16, tag="ns1")
            nc.scalar.activation(s1[:], TA[:], AF.Silu)
            h2t = pw.tile([32, 10], F32, tag="w")
            nc.tensor.matmul(h2t[:], W2[:], s1[:], start=True, stop=True)
            s2 = sp.tile([32, 10], BF16, tag="ns2")
            nc.scalar.activation(s2[:], h2t[:], AF.Silu)
            atp = pw.tile([10, 8], F32, tag="w")
            nc.tensor.matmul(atp[:], s2[:], W3[:], start=True, stop=True)
            AiTab = sp.tile([10, 8], BF16, tag="nat")
            nc.scalar.copy(AiTab[:], atp[:])
            Tsb = bp.tile([P, NCH * P], BF16)
            j = 0
            while j * 896 < NCH * P:
                s = j * 896
                n = min(896, NCH * P - s)
                ncc = n // P
                aiT = pf.tile([P, 7 * 8], F32, tag="f")
                for c in range(ncc):
                    nc.tensor.matmul(aiT[:, c * 8:(c + 1) * 8],
                                     ohA[:, s + c * P:s + (c + 1) * P],
                                     AiTab[:], start=True, stop=True)
                f16a = sp.tile([P, 7 * 8], BF16, tag="f16a")
                nc.scalar.copy(f16a[:, 0:ncc * 8], aiT[:, 0:ncc * 8])
                nc.vector.tensor_copy(
                    Tsb[:, s:s + n].rearrange("p (k r v) -> p k r v", v=8, r=16),
                    f16a[:, 0:ncc * 8].rearrange("p (k v) -> p k v", v=8)
                    .unsqueeze(2).to_broadcast([P, ncc, 16, 8]))
                j += 1

            # ---------------- geometry (plane layout, whole E) ----------------
            gm = bp.tile([P, 18 * C], F32)
            nc.sync.dma_start(gm[:], geom_d[:])
            g3 = gm[:].rearrange("p (f c) -> p f c", f=18)
            tmp9 = bp.tile([P, 9 * C], F32)
            nc.vector.tensor_tensor(
                out=tmp9[:].rearrange("p (i j c) -> p i j c", i=3, j=3),
                in0=gm[:, 9 * C:18 * C].rearrange("p (i j c) -> p i j c", i=3, j=3),
                in1=gm[:, 6 * C:9 * C].rearrange("p (i c) -> p i c", i=3)
                    .unsqueeze(2).to_broadcast([P, 3, 3, C]),
                op=ALU.mult)
            sv = bp.tile([P, 3 * C], F32)
            nc.vector.tensor_tensor(out=sv[:], in0=tmp9[:, 0:3 * C],
                                    in1=tmp9[:, 3 * C:6 * C], op=ALU.add)
            nc.vector.tensor_tensor(out=sv[:], in0=sv[:],
                                    in1=tmp9[:, 6 * C:9 * C], op=ALU.add)
            ev = bp.tile([P, 3 * C], F32)
            nc.vector.tensor_tensor(out=ev[:], in0=g3[:, 3:6].rearrange("p f c -> p (f c)"),
                                    in1=g3[:, 0:3].rearrange("p f c -> p (f c)"),
                                    op=ALU.subtract)
            nc.vector.tensor_tensor(out=ev[:], in0=ev[:], in1=sv[:], op=ALU.add)
            sq = bp.tile([P, 3 * C], F32)
            nc.gpsimd.tensor_tensor(out=sq[:], in0=ev[:], in1=ev[:], op=ALU.mult)
            ln2 = bp.tile([P, C], F32)
            nc.vector.tensor_tensor(out=ln2[:], in0=sq[:, 0:C], in1=sq[:, C:2 * C],
                                    op=ALU.add)
            nc.vector.tensor_tensor(out=ln2[:], in0=ln2[:], in1=sq[:, 2 * C:3 * C],
                                    op=ALU.add)
            ln = bp.tile([P, C], F32)
            nc.scalar.activation(ln[:], ln2[:], AF.Sqrt)
            rl = bp.tile([P, C], F32)
            nc.vector.reciprocal(rl[:], ln[:])
            u = bp.tile([P, 3 * C], F32)
            nc.vector.tensor_tensor(
                out=u[:].rearrange("p (f c) -> p f c", f=3),
                in0=ev[:].rearrange("p (f c) -> p f c", f=3),
                in1=rl[:].unsqueeze(1).to_broadcast([P, 3, C]), op=ALU.mult)
            usq = bp.tile([P, 3 * C], F32)
            nc.gpsimd.tensor_tensor(out=usq[:], in0=u[:], in1=u[:], op=ALU.mult)
            # feature planes: f-major [basis10 | u 3 | sh2 5 | ones]
            gf = bp.tile([P, NF * C], BF16)
            dt2 = bp.tile([P, 10 * C], F32)
            for b in range(N_BASIS):
                nc.scalar.activation(dt2[:, b * C:(b + 1) * C], ln[:], AF.Square,
                                     bias=cv[:, b:b + 1],
                                     scale=cv[:, 10:11])
            nc.scalar.activation(gf[:, 0:10 * C], dt2[:], AF.Exp,
                                 scale=cv[:, 11:12])
            nc.vector.tensor_copy(gf[:, 10 * C:13 * C], u[:])
            t1 = bp.tile([P, C], F32)
            nc.scalar.mul(t1[:], u[:, 2 * C:3 * C], cv[:, 12:13])       # sqrt15*uz
            nc.gpsimd.tensor_tensor(out=gf[:, 13 * C:14 * C], in0=u[:, 0:C],
                                    in1=t1[:], op=ALU.mult)     # m0
            nc.gpsimd.tensor_tensor(out=gf[:, 16 * C:17 * C], in0=u[:, C:2 * C],
                                    in1=t1[:], op=ALU.mult)     # m3
            nc.scalar.mul(t1[:], u[:, 0:C], cv[:, 12:13])               # sqrt15*ux
            nc.gpsimd.tensor_tensor(out=gf[:, 14 * C:15 * C], in0=u[:, C:2 * C],
                                    in1=t1[:], op=ALU.mult)     # m1
            t2 = bp.tile([P, C], F32)
            nc.vector.tensor_tensor(out=t2[:], in0=usq[:, 0:C],
                                    in1=usq[:, 2 * C:3 * C], op=ALU.add)
            nc.scalar.mul(t2[:], t2[:], cv[:, 13:14])
            t3 = bp.tile([P, C], F32)
            nc.scalar.mul(t3[:], usq[:, C:2 * C], cv[:, 14:15])
            nc.vector.tensor_tensor(out=gf[:, 15 * C:16 * C], in0=t3[:], in1=t2[:],
                                    op=ALU.subtract)            # m2
            nc.vector.tensor_tensor(out=t2[:], in0=usq[:, 2 * C:3 * C],
                                    in1=usq[:, 0:C], op=ALU.subtract)
            nc.scalar.mul(gf[:, 17 * C:18 * C], t2[:], cv[:, 15:16])  # m4
            nc.gpsimd.memset(gf[:, 18 * C:19 * C], 1.0)               # ones
            gfv = gf[:].rearrange("p (f c) -> p f c", f=NF)

            NSL = [(0, 512), (512, ET)] if ET > 512 else [(0, ET)]
            # ---------------- edge tiles (software-pipelined) ----------------
            # front(t): gathers + geometry transpose + radial MLP (PE+Act)
            # back(t-1): Sel/W4p tensor product + scatter (PE+DVE)
            state = {"win_ps": None}

            def stageB(t):
                wcols = slice(t * (NIW // NT), (t + 1) * (NIW // NT))
                aiS = sp.tile([P, ET], BF16, tag="aiS", bufs=3)
                nc.gpsimd.dma_gather(
                    aiS[:].unsqueeze(1), Tsb[:, :], isrc[:, wcols], ET, ET, P,
                    transpose=True, sbuf_tokens_per_rank=128,
                    sbuf_free_dim_per_rank=256)
                aiD = sp.tile([P, ET], BF16, tag="aiD")
                nc.gpsimd.dma_gather(
                    aiD[:].unsqueeze(1), Tsb[:, :], idst[:, wcols], ET, ET, P,
                    transpose=True, sbuf_tokens_per_rank=128,
                    sbuf_free_dim_per_rank=256)
                oht = sp.tile([P, ET], BF16, tag="oht", bufs=4)
                nc.sync.dma_start(oht[:], ohm_d[:, t * ET:(t + 1) * ET])
                # batched transpose of geometry features -> [19, ET]
                btp = pf.tile([NF, ET], BF16, tag="f", name="btp")
                for cc in range(TILE_CH):
                    cg = t * TILE_CH + cc
                    nc.tensor.transpose(btp[:, cc * P:(cc + 1) * P],
                                        gfv[:, 0:NF, cg], identb[:])
                bt19 = sp.tile([NF, ET], BF16, tag="bt19")
                nc.vector.tensor_copy(bt19[:], btp[:])
                # radial MLP layer 1 and sh replication (separate PSUM
                # slots so each slot's recycle waits one Act op, not two)
                h1p = ps.tile([64, ET], F32, tag="s", name="h1p")
                for a, b in NSL:
                    nc.tensor.matmul(h1p[:, a:b], HSt[:, 0:64], bt19[:, a:b],
                                     start=True, stop=True)
                shp = ps.tile([60, ET], F32, tag="s", name="shp")
                for a, b in NSL:
                    nc.tensor.matmul(shp[:, a:b], HSt[:, 64:124], bt19[:, a:b],
                                     start=True, stop=True)
                h1 = sp.tile([64, ET], BF16, tag="eh1")
                nc.scalar.activation(h1[:], h1p[:], AF.Silu)
                shs = sp.tile([60, ET], BF16, tag="shs", bufs=3)
                nc.scalar.copy(shs[:], shp[:])
                h2p = ps.tile([64, ET], F32, tag="s")
                for a, b in NSL:
                    nc.tensor.matmul(h2p[:, a:b], fW2[:], h1[:, a:b],
                                     start=True, stop=True)
                h2 = sp.tile([64, ET], BF16, tag="eh2")
                nc.scalar.activation(h2[:], h2p[:], AF.Silu)
                return dict(t=t, aiS=aiS, aiD=aiD, oht=oht, shs=shs, h2=h2)

            def cps_partial(cps_h, m, q, rq, a, b):
                nc.tensor.matmul(cps_h[:, 0:b - a],
                                 W4pt[:, q * 224 + m * 112:
                                      q * 224 + (m + 1) * 112],
                                 rq[:, a:b], start=(q == 0), stop=(q == 3))

            def stageC1(cur):
                aiD, h2 = cur["aiD"], cur["h2"]
                # layer-3 matmul pre-expanded by Sel (silu commutes with the
                # 0/1 row selection); rq TT runs all-bf16 at 2x DVE rate
                rqs = []
                for q in range(4):
                    wrp = ps.tile([P, ET], F32, tag="s")
                    for a, b in NSL:
                        nc.tensor.matmul(wrp[:, a:b],
                                         fW3[:, 128 * q:128 * (q + 1)],
                                         h2[:, a:b], start=True, stop=True)
                    wrpS = sp.tile([P, ET], BF16, tag=f"wrpS{q}")
                    nc.scalar.activation(wrpS[:], wrp[:], AF.Silu)
                    rq = sp.tile([P, ET], BF16, tag=f"rq{q}")
                    nc.vector.tensor_tensor(out=rq[:], in0=wrpS[:], in1=aiD[:],
                                            op=ALU.mult)
                    rqs.append(rq)
                cur["rqs"] = rqs

            def stageC2(cur):
                aiS, rqs = cur["aiS"], cur["rqs"]
                tm0 = sp.tile([112, ET], BF16, tag="tm0")
                for hi, (a, b) in enumerate(NSL):
                    cps0h = pc.tile([112, b - a], F32, tag=f"c{hi}",
                                    name="cps0h")
                    for q in range(4):
                        cps_partial(cps0h, 0, q, rqs[q], a, b)
                    nc.vector.tensor_tensor(out=tm0[:, a:b],
                                            in0=cps0h[:, 0:b - a],
                                            in1=aiS[0:112, a:b], op=ALU.mult)
                cur["tm0"] = tm0

            def stageD(cur):
                aiS, shs = cur["aiS"], cur["shs"]
                tm1 = sp.tile([112, ET], BF16, tag="tm1")
                fps = ps.tile([60, ET], F32, tag="s")
                F = sp.tile([60, ET], BF16, tag="F")
                for hi, (a, b) in enumerate(NSL):
                    cps1h = pc.tile([112, b - a], F32, tag=f"c{hi}",
                                    name="cps1h")
                    for q in range(4):
                        cps_partial(cps1h, 1, q, cur["rqs"][q], a, b)
                    nc.vector.tensor_tensor(out=tm1[:, a:b],
                                            in0=cps1h[:, 0:b - a],
                                            in1=aiS[0:112, a:b], op=ALU.mult)
                    nc.tensor.matmul(fps[:, a:b], L2At[:], cur["tm0"][:, a:b],
                                     start=True, stop=False)
                    nc.tensor.matmul(fps[:, a:b], L2Bt[:], tm1[:, a:b],
                                     start=False, stop=True)
                    nc.vector.tensor_tensor(out=F[:, a:b], in0=fps[:, a:b],
                                            in1=shs[:, a:b], op=ALU.mult)
                cur["F"] = F

            def stageE(cur):
                t, oht, F = cur["t"], cur["oht"], cur["F"]
                # scatter: transpose each chunk, one copy, PSUM-accum matmuls
                ftp = pf.tile([P, TILE_CH * 60], BF16, tag="f",
                              name="ftp")
                fc = sp.tile([P, TILE_CH * 60], BF16, tag="fc")
                hsplit = min(4, TILE_CH)
                for cc in range(hsplit):
                    nc.tensor.transpose(ftp[:, cc * 60:(cc + 1) * 60],
                                        F[:, cc * P:(cc + 1) * P],
                                        identb[0:60, 0:60])
                nc.vector.tensor_copy(fc[:, 0:hsplit * 60],
                                      ftp[:, 0:hsplit * 60])
                for cc in range(hsplit, TILE_CH):
                    nc.tensor.transpose(ftp[:, cc * 60:(cc + 1) * 60],
                                        F[:, cc * P:(cc + 1) * P],
                                        identb[0:60, 0:60])
                if TILE_CH > hsplit:
                    nc.vector.tensor_copy(fc[:, hsplit * 60:],
                                          ftp[:, hsplit * 60:])
                for cc in range(TILE_CH):
                    cg = t * TILE_CH + cc
                    win = cg // W_CH
                    if cg % W_CH == 0:
                        state["win_ps"] = pw.tile([P, 60], F32, tag="w",
                                                  name="win_ps")
                    nc.tensor.matmul(state["win_ps"][:],
                                     oht[:, cc * P:(cc + 1) * P],
                                     fc[:, cc * 60:(cc + 1) * 60],
                                     start=(cg % W_CH == 0),
                                     stop=(cg % W_CH == W_CH - 1))
                    if cg % W_CH == W_CH - 1:
                        wsb = sp.tile([P, 60], F32, tag="wsb")
                        nc.scalar.copy(wsb[:], state["win_ps"][:])
                        nc.sync.dma_start(out_d[win * P:(win + 1) * P, :],
                                          wsb[:])

            tiles = {}
            for i in range(NT + 3):
                if i < NT:
                    tiles[i] = stageB(i)
                if i - 1 >= 0 and i - 1 < NT:
                    stageC1(tiles[i - 1])
                if i - 3 >= 0:
                    stageE(tiles.pop(i - 3))
                if i - 2 >= 0 and i - 2 < NT:
                    stageD(tiles[i - 2])
                if i - 1 >= 0 and i - 1 < NT:
                    stageC2(tiles[i - 1])
    nc.compile()
    return nc


_CACHE = {}


def kernel(**inputs):
    per_core, onehotA, W_CH, C_TOT, E = _host_prep(inputs)
    et = np.asarray(inputs["embed_table"], np.float32)
    fW4 = np.asarray(inputs["fc_W4"], np.float32)
    consts = _build_consts(fW4)
    W4p, Sel, L2A, L2B, HSb = consts
    HS = _merge_hs(HSb, np.asarray(inputs["fc_W1"], np.float32) / 1.12)
    key = (W_CH, C_TOT)
    if key not in _CACHE:
        _CACHE[key] = _build_bass(W_CH, C_TOT, E, consts)
    nc = _CACHE[key]
    shared = dict(
        onehotA=onehotA,
        TA=np.ascontiguousarray(
            (et @ np.asarray(inputs["fit_W1"], np.float32)).T),
        fit_W2=np.asarray(inputs["fit_W2"], np.float32),
        fit_W3=np.asarray(inputs["fit_W3"], np.float32),
        HS=HS,
        fc_W2p=(np.asarray(inputs["fc_W2"], np.float32) / 8.0),
        fc_W3p=np.ascontiguousarray(np.concatenate(
            [(np.asarray(inputs["fc_W3"], np.float32) / 8.0) @ Sel[q]
             for q in range(4)], axis=1)),
        W4p=np.ascontiguousarray(np.transpose(W4p.reshape(4, 128, 224), (1, 0, 2)).reshape(128, 896)),
        cvec=np.tile(np.array([*(-VCENTERS / STEP), 1.0 / STEP, -1.0,
                               15.0 ** 0.5, 0.5 * 5.0 ** 0.5, 5.0 ** 0.5,
                               0.5 * 15.0 ** 0.5], np.float32), (P, 1)),
        L2A=L2A, L2B=L2B,
    )
    import ml_dtypes
    for k in ("W4p", "L2A", "L2B", "HS", "TA", "fit_W2", "fit_W3",
              "fc_W2p", "fc_W3p", "onehotA"):
        shared[k] = shared[k].astype(ml_dtypes.bfloat16)
    in_maps = []
    for ci in range(N_CORES):
        m = dict(shared)
        m.update(geom_pl=per_core[ci]["geom_pl"],
                 oh_pl=per_core[ci]["oh_pl"].astype(ml_dtypes.bfloat16),
                 idx_src=per_core[ci]["idx_src"], idx_dst=per_core[ci]["idx_dst"])
        in_maps.append(m)
    res = bass_utils.run_bass_kernel_spmd(nc, in_maps, core_ids=list(range(N_CORES)))
    out = np.concatenate([res.results[ci]["out"][:NPC] for ci in range(N_CORES)], 0)
    return out.astype(np.float32)



# revision 2
# speedup vs baseline: 1.4453x; 1.4453x over previous
"""E3Conv Trainium2 kernel: 8-core SPMD, dst-partitioned edges.

Core i owns nodes [1250i,1250(i+1)) and all edges into them (no collective).
Host precomputes per-edge geometry features (radial basis, spherical-harmonic
planes) and per-edge node-scalar gathers (node MLP degenerates to a 10-row
atom-type table), laid out as [rows, E] bf16 planes streamed per tile.
Device per 896-edge tile: radial MLP (PE matmul + Act silu), quadrant
tensor-product contraction (PE, f32 PSUM), aiS/aiD multiplies (DVE/Pool),
L2 reduction, transpose, and dma_scatter_add straight into an HBM
accumulator. Edges are host-packed so each 512/384-token scatter span has
unique destinations (CCE-add races within one DMA, accumulates across DMAs).
Scatter-mean division by counts happens on host.
"""
import sys
sys.path.insert(0, "/opt/trn_rl_repo")
import numpy as np

import concourse.bass as bass
import concourse.tile as tile
from concourse import bacc, mybir
from concourse import bass_utils
from concourse.masks import make_identity

P = 128
N_NODES, N_EDGES, N_GRAPHS = 10000, 131072, 64
N_CORES, NPC = 8, 1250
OUTR = 1280              # out rows per core (1250 real + junk row 1250)
JUNK = NPC               # scatter row for padded tokens
MAX_RADIUS, N_BASIS = 4.0, 10
STEP = MAX_RADIUS / (N_BASIS + 1)
VCENTERS = np.linspace(0.0, MAX_RADIUS, N_BASIS + 2)[1:-1].astype(np.float32)
F32, BF16, I16 = mybir.dt.float32, mybir.dt.bfloat16, mybir.dt.int16
AF = mybir.ActivationFunctionType
ALU = mybir.AluOpType
TILE_CH = 7
ET = TILE_CH * P         # 896 edges per tile
NSL = [(0, 512), (512, ET)]
SPAN_CH = (4, 3)         # scatter spans per tile: 512 + 384 tokens


def _silu(x):
    return x / (1.0 + np.exp(-x))


def _build_consts(fW3, fW4):
    """W4p [128,4*224] quadrant-stationary layout, Sel-fused fW3p [64,512],
    L2A/L2B [112,60] u-reduction matrices."""
    s3 = 3.0 ** 0.5
    W4p = np.zeros((512, 224), np.float32)
    offs = {0: 0, 1: 1024, 2: 1536}
    wbase = {0: 0, 1: 16, 2: 24}
    scale_l = {0: 1.0 / 64, 1: s3 / 64, 2: 1.0 / 64}
    for l, mo in enumerate((16, 8, 4)):
        for u in range(8):
            for v in range(8):
                for wl in range(mo):
                    col = offs[l] + (u * 8 + v) * mo + wl
                    w = wbase[l] + wl
                    W4p[np.arange(64) * 8 + v, w * 8 + u] = fW4[:, col] * scale_l[l]
    W4pt = np.ascontiguousarray(
        np.transpose(W4p.reshape(4, 128, 224), (1, 0, 2)).reshape(128, 896))
    Sel = np.zeros((4, 64, 128), np.float32)
    for q in range(4):
        for r in range(128):
            Sel[q, 16 * q + r // 8, r] = 1.0
    fW3p = np.ascontiguousarray(np.concatenate(
        [(fW3 / 8.0) @ Sel[q] for q in range(4)], axis=1))  # [64, 512]
    L2A = np.zeros((112, 60), np.float32)
    L2B = np.zeros((112, 60), np.float32)
    for r in range(112):
        L2A[r, r // 8] = 1.0
        w = 14 + r // 8
        if w < 16:
            L2B[r, w] = 1.0
        elif w < 24:
            for m in range(3):
                L2B[r, 16 + (w - 16) * 3 + m] = 1.0
        else:
            for k in range(5):
                L2B[r, 40 + (w - 24) * 5 + k] = 1.0
    return W4pt, fW3p, L2A, L2B


def _pack_spans(dstl, C_TOT):
    """Assign each edge to a scatter span so no span repeats a destination.
    Spans alternate capacity 512/384 (chunk-aligned halves of each tile).
    Returns token->edge map [E] (-1 = pad) or None if infeasible."""
    NT = C_TOT // TILE_CH
    nspan = 2 * NT
    caps = np.where(np.arange(nspan) % 2 == 0, SPAN_CH[0] * P, SPAN_CH[1] * P)
    load = np.zeros(nspan, np.int64)
    fill = [[] for _ in range(nspan)]
    order = np.argsort(dstl, kind="stable")
    bounds = np.searchsorted(dstl[order], np.arange(NPC + 1))
    groups = [(bounds[n + 1] - bounds[n], order[bounds[n]:bounds[n + 1]])
              for n in range(NPC)]
    groups.sort(key=lambda g: -g[0])
    for c, elist in groups:
        if c == 0:
            break
        if c > nspan:
            return None
        rem = caps - load
        sel = np.argpartition(-rem, c - 1)[:c]
        if rem[sel].min() <= 0:
            return None
        for s, e in zip(sel, elist):
            fill[s].append(e)
            load[s] += 1
    tok = np.full(C_TOT * P, -1, np.int64)
    for s in range(nspan):
        t, half = s // 2, s % 2
        start = t * ET + (0 if half == 0 else SPAN_CH[0] * P)
        tok[start:start + load[s]] = fill[s]
    return tok


def _host_prep(inputs):
    pos = np.asarray(inputs["pos"], np.float32)
    A = np.asarray(inputs["A"]).astype(np.int64)
    batch = np.asarray(inputs["batch"]).astype(np.int64)
    esrc = np.asarray(inputs["edge_src"]).astype(np.int64)
    edst = np.asarray(inputs["edge_dst"]).astype(np.int64)
    shifts = np.asarray(inputs["edge_shifts"], np.float32)
    cell = np.asarray(inputs["cell"], np.float32)
    counts = np.bincount(edst, minlength=N_NODES).astype(np.float32)
    cpn = cell[batch]                                   # [N,3,3]

    # node MLP is atom-type degenerate: 10-row table on host
    et = np.asarray(inputs["embed_table"], np.float32)
    h = _silu(et @ np.asarray(inputs["fit_W1"], np.float32)
              + np.asarray(inputs["fit_b1"], np.float32))
    h = _silu(h @ np.asarray(inputs["fit_W2"], np.float32)
              + np.asarray(inputs["fit_b2"], np.float32))
    AiTab = (h @ np.asarray(inputs["fit_W3"], np.float32)
             + np.asarray(inputs["fit_b3"], np.float32))  # [10, 8]
    AiA = AiTab[A]                                        # [N, 8]

    core_edges = []
    cmax = 0
    for ci in range(N_CORES):
        lo = ci * NPC
        ids = np.nonzero((edst >= lo) & (edst < lo + NPC))[0]
        core_edges.append(ids)
        cmax = max(cmax, (len(ids) + P - 1) // P)
    C_TOT = ((cmax + TILE_CH - 1) // TILE_CH) * TILE_CH
    toks = None
    while toks is None:
        toks = []
        for ci in range(N_CORES):
            t = _pack_spans(edst[core_edges[ci]] - ci * NPC, C_TOT)
            if t is None:
                toks = None
                C_TOT += TILE_CH
                break
            toks.append(t)
    E = C_TOT * P

    import ml_dtypes
    per_core = []
    for ci in range(N_CORES):
        ids = core_edges[ci]
        tok = toks[ci]                      # [E] -> index into ids, or -1
        pad = tok < 0
        e_ids = np.where(pad, 0, ids[np.maximum(tok, 0)])
        src = np.where(pad, 0, esrc[e_ids])
        dstg = np.where(pad, 0, edst[e_ids])
        dstl = np.where(pad, JUNK, dstg - ci * NPC).astype(np.int16)
        sh = np.where(pad[:, None], 0.0, shifts[e_ids]).astype(np.float32)
        sv = np.einsum('ei,eij->ej', sh, cpn[src])
        ev = pos[dstg] - pos[src] + sv
        L = np.sqrt((ev * ev).sum(1))
        u = ev / np.maximum(L, 1e-9)[:, None]
        x, y, z = u[:, 0], u[:, 1], u[:, 2]
        s5, s15 = 5.0 ** 0.5, 15.0 ** 0.5
        sh2 = np.stack([s15 * x * z, s15 * x * y,
                        s5 * (y * y - 0.5 * (x * x + z * z)),
                        s15 * y * z, 0.5 * s15 * (z * z - x * x)], -1)
        diff = (L[:, None] - VCENTERS) / STEP
        bas = np.exp(-diff * diff) / 1.12                # [E,10]
        shs = np.empty((60, E), np.float32)
        shs[0:16] = 1.0
        for w in range(8):
            shs[16 + 3 * w:19 + 3 * w] = u.T
        for w in range(4):
            shs[40 + 5 * w:45 + 5 * w] = sh2.T
        wr = dstl.reshape(-1, 16).T                      # [16, E/16]
        per_core.append(dict(
            bas=bas.T.astype(ml_dtypes.bfloat16),
            shs=shs.astype(ml_dtypes.bfloat16),
            aiS=np.ascontiguousarray(
                np.tile(AiA[src].T, (16, 1))).astype(ml_dtypes.bfloat16),
            aiD=np.ascontiguousarray(
                np.tile(AiA[dstg].T, (16, 1))).astype(ml_dtypes.bfloat16),
            idx=np.ascontiguousarray(np.tile(wr, (8, 1))),
        ))
    return per_core, counts, C_TOT


def _build_bass(C_TOT):
    NT = C_TOT // TILE_CH
    E = C_TOT * P
    nc = bacc.Bacc("TRN2", target_bir_lowering=False, debug=False,
                   num_devices=N_CORES)

    def din(name, shape, dt=BF16):
        return nc.dram_tensor(name, shape, dt, kind="ExternalInput").ap()

    bas_d = din("bas", [10, E])
    shs_d = din("shs", [60, E])
    aiS_d = din("aiS", [P, E])
    aiD_d = din("aiD", [P, E])
    idx_d = din("idx", [P, E // 16], I16)
    fW1_d = din("fW1p", [10, 64])
    fW2_d = din("fW2p", [64, 64])
    fW3_d = din("fW3p", [64, 512])
    W4_d = din("W4pt", [P, 896])
    L2A_d = din("L2A", [112, 60])
    L2B_d = din("L2B", [112, 60])
    out_d = nc.dram_tensor("out", [OUTR, 64], F32, kind="ExternalOutput").ap()

    with tile.TileContext(nc) as tc:
        with tc.tile_pool(name="const", bufs=1) as cp, \
             tc.tile_pool(name="sb", bufs=2) as sp, \
             tc.tile_pool(name="inp", bufs=3) as ip, \
             tc.tile_pool(name="big", bufs=2, space="PSUM") as pb, \
             tc.tile_pool(name="pc", bufs=1, space="PSUM") as pc, \
             tc.tile_pool(name="pt", bufs=1, space="PSUM") as pt:
            ident = cp.tile([P, P], F32)
            make_identity(nc, ident[:])
            identb = cp.tile([P, P], BF16)
            nc.vector.tensor_copy(identb[:], ident[:])

            def load_const(dram, shape, dt=BF16):
                t = cp.tile(shape, dt, tag=dram.tensor.name)
                nc.sync.dma_start(t[:], dram[:])
                return t
            fW1p = load_const(fW1_d, [10, 64])
            fW2p = load_const(fW2_d, [64, 64])
            fW3p = load_const(fW3_d, [64, 512])
            W4pt = load_const(W4_d, [P, 896])
            L2At = load_const(L2A_d, [112, 60])
            L2Bt = load_const(L2B_d, [112, 60])
            idx = cp.tile([P, E // 16], I16, tag="idx")
            nc.sync.dma_start(idx[:], idx_d[:])

            # zero the HBM accumulator (scatters are WAW-ordered after this)
            zsb = cp.tile([P, OUTR * 64 // P], F32, tag="zsb")
            nc.gpsimd.memset(zsb[:], 0.0)
            nc.sync.dma_start(
                out_d[:].rearrange("(c p) e -> p c e", p=P),
                zsb[:].rearrange("p (c e) -> p c e", e=64))

            def stF(t):
                sl = slice(t * ET, (t + 1) * ET)
                bas = ip.tile([10, ET], BF16, tag="bas")
                nc.sync.dma_start(bas[:], bas_d[:, sl])
                shs = ip.tile([60, ET], BF16, tag="shs", bufs=4)
                nc.sync.dma_start(shs[:], shs_d[:, sl])
                aiS = ip.tile([P, ET], BF16, tag="aiS", bufs=4)
                nc.sync.dma_start(aiS[:], aiS_d[:, sl])
                aiD = ip.tile([P, ET], BF16, tag="aiD")
                nc.sync.dma_start(aiD[:], aiD_d[:, sl])
                h1p = pb.tile([P, ET], F32, tag="big", name="h1p")
                for a, b in NSL:
                    nc.tensor.matmul(h1p[0:64, a:b], fW1p[:], bas[:, a:b],
                                     start=True, stop=True)
                h1 = sp.tile([64, ET], BF16, tag="h1")
                nc.scalar.activation(h1[:], h1p[0:64, :], AF.Silu)
                h2p = pb.tile([P, ET], F32, tag="big", name="h2p")
                for a, b in NSL:
                    nc.tensor.matmul(h2p[0:64, a:b], fW2p[:], h1[:, a:b],
                                     start=True, stop=True)
                h2 = sp.tile([64, ET], BF16, tag="h2")
                nc.scalar.activation(h2[:], h2p[0:64, :], AF.Silu)
                return dict(t=t, shs=shs, aiS=aiS, aiD=aiD, h2=h2)

            def stQ(cur):
                h2, aiD = cur["h2"], cur["aiD"]
                rqs = []
                for q in range(4):
                    wrp = pb.tile([P, ET], F32, tag="big", name=f"wrp{q}")
                    for a, b in NSL:
                        nc.tensor.matmul(wrp[:, a:b],
                                         fW3p[:, q * 128:(q + 1) * 128],
                                         h2[:, a:b], start=True, stop=True)
                    wS = sp.tile([P, ET], BF16, tag=f"wS{q}")
                    nc.scalar.activation(wS[:], wrp[:], AF.Silu)
                    rq = sp.tile([P, ET], BF16, tag=f"rq{q}")
                    if q == 0:
                        nc.gpsimd.tensor_tensor(out=rq[:], in0=wS[:],
                                                in1=aiD[:], op=ALU.mult)
                    else:
                        nc.vector.tensor_tensor(out=rq[:], in0=wS[:],
                                                in1=aiD[:], op=ALU.mult)
                    rqs.append(rq)
                cur["rqs"] = rqs

            QORD = (1, 2, 3, 0)   # q0's rq comes from slow Pool: contract last

            def stM(cur):
                rqs, aiS = cur["rqs"], cur["aiS"]
                tms = []
                for m in range(2):
                    ch = [pc.tile([112, 512], F32, tag="c0", name=f"cp{m}h0"),
                          pc.tile([112, 384], F32, tag="c1", name=f"cp{m}h1")]
                    for qi, q in enumerate(QORD):
                        for hi, (a, b) in enumerate(NSL):
                            nc.tensor.matmul(
                                ch[hi][:, 0:b - a],
                                W4pt[:, q * 224 + m * 112:
                                     q * 224 + (m + 1) * 112],
                                rqs[q][:, a:b], start=(qi == 0), stop=(qi == 3))
                    tm = sp.tile([112, ET], BF16, tag=f"tm{m}")
                    for hi, (a, b) in enumerate(NSL):
                        nc.vector.tensor_tensor(out=tm[:, a:b],
                                                in0=ch[hi][:, 0:b - a],
                                                in1=aiS[0:112, a:b],
                                                op=ALU.mult)
                    tms.append(tm)
                cur["tms"] = tms

            def stB1(cur):
                tms = cur["tms"]
                fps = [pc.tile([112, 512], F32, tag="c0", name="fps0"),
                       pc.tile([112, 384], F32, tag="c1", name="fps1")]
                for hi, (a, b) in enumerate(NSL):
                    nc.tensor.matmul(fps[hi][0:60, 0:b - a], L2At[:],
                                     tms[0][:, a:b], start=True, stop=False)
                    nc.tensor.matmul(fps[hi][0:60, 0:b - a], L2Bt[:],
                                     tms[1][:, a:b], start=False, stop=True)
                cur["fps"] = fps

            def stB2a(cur):
                fps, shs = cur["fps"], cur["shs"]
                F = sp.tile([60, ET], BF16, tag="F")
                for hi, (a, b) in enumerate(NSL):
                    nc.vector.tensor_tensor(out=F[:, a:b],
                                            in0=fps[hi][0:60, 0:b - a],
                                            in1=shs[:, a:b], op=ALU.mult)
                cur["F"] = F

            def stB2b(cur):
                t, F = cur["t"], cur["F"]
                ftp = pt.tile([P, TILE_CH * 60], BF16, tag="ftp", name="ftp")
                for cc in range(TILE_CH):
                    nc.tensor.transpose(ftp[:, cc * 60:(cc + 1) * 60],
                                        F[:, cc * P:(cc + 1) * P],
                                        identb[0:60, 0:60])
                fc = sp.tile([P, TILE_CH * 64], F32, tag="fc")
                fc3 = fc[:].rearrange("p (c e) -> p c e", e=64)
                nc.vector.tensor_copy(
                    fc3[:, :, 0:60],
                    ftp[:].rearrange("p (c e) -> p c e", e=60))
                c0 = 0
                for s, sch in enumerate(SPAN_CH):
                    n = sch * P
                    icol = t * (ET // 16) + c0 * P // 16
                    nc.gpsimd.dma_scatter_add(
                        out_d[:], fc3[:, c0:c0 + sch, :],
                        idx[:, icol:icol + n // 16], n, n, 64)
                    c0 += sch

            # software pipeline: B1/B2(i-2) feed PE/DVE early, front(i)
            # keeps the Act silu chain fed, mid(i-1) fills PE.
            tiles = {}
            for i in range(NT + 2):
                if i - 2 >= 0:
                    stB1(tiles[i - 2])
                if i < NT:
                    tiles[i] = stF(i)
                if i - 2 >= 0:
                    stB2a(tiles[i - 2])
                if i < NT:
                    stQ(tiles[i])
                if i - 2 >= 0:
                    stB2b(tiles.pop(i - 2))
                if i - 1 >= 0 and i - 1 < NT:
                    stM(tiles[i - 1])
    nc.compile()
    return nc


_CACHE = {}


def kernel(**inputs):
    per_core, counts, C_TOT = _host_prep(inputs)
    W4pt, fW3p, L2A, L2B = _build_consts(
        np.asarray(inputs["fc_W3"], np.float32),
        np.asarray(inputs["fc_W4"], np.float32))
    if C_TOT not in _CACHE:
        _CACHE[C_TOT] = _build_bass(C_TOT)
    nc = _CACHE[C_TOT]
    import ml_dtypes
    shared = dict(
        fW1p=np.asarray(inputs["fc_W1"], np.float32),
        fW2p=np.asarray(inputs["fc_W2"], np.float32) / 8.0,
        fW3p=fW3p, W4pt=W4pt, L2A=L2A, L2B=L2B,
    )
    for k in shared:
        shared[k] = shared[k].astype(ml_dtypes.bfloat16)
    in_maps = []
    for ci in range(N_CORES):
        m = dict(shared)
        m.update(per_core[ci])
        in_maps.append(m)
    res = bass_utils.run_bass_kernel_spmd(nc, in_maps,
                                          core_ids=list(range(N_CORES)))
    out = np.concatenate([res.results[ci]["out"][:NPC, :60]
                          for ci in range(N_CORES)], 0)
    return (out / np.maximum(counts, 1.0)[:, None]).astype(np.float32)
